# revision 11
# baseline (speedup 1.0000x reference)
"""BFP (block floating point) activation quantization kernel for Trainium2.

Problem: NCHW input [32, 256, 56, 56] f32. Blocks of 8 consecutive channels
share one exponent (at each (n, h, w) position). Per block:
    maxabs = max |x_i|
    p      = 2^floor(log2(maxabs))       (power-of-two part of maxabs)
    s      = p / 4                       (scale; mantissa_bits = 3)
    q_i    = clip(round_half_even(x_i/s), -7, 7) * s

Distribution: batch dim sharded 4 per core across 8 cores; per core the
SBUF partition dim is (n, cb) = 4 batches x 32 channel-blocks = 128, free
dims are (ch in [0,8), spatial chunk).

Device pipeline (all per-element math in f16; exact relative to f16(x)):
    a16  = |f16(x)|              ACT Abs pass (the only ACT use)
    m16  = tree-max over ch      3 packed-f16 DVE TT max passes (4+2+1)
    pbh  = m16 & 0x7C00          f16 power of two = 2^floor(log2(maxabs))
    invh = 2^-e  (bits 0x7800 - pbh), invh4 = 4*invh = 2^(2-e)
    r4   = f16(x) * invh4        f16 TT (== f16(x * 2^(2-e)) exactly:
                                 power-of-2 scaling commutes with rounding)
    w8   = int8(r4)              ONE ACT Copy pass with int8 output: the
                                 conversion rounds to nearest-even
                                 (HW-verified), putting round+convert on
                                 the otherwise-idle ACT engine; the host
                                 clips the resulting [-8, 8] to [-7, 7]
Outputs: w8 (int8 mantissas, 25.7MB) + pbh (f16 block scales, 3.2MB).
The host reconstructs q = w8 * (pbh/4) in f32 — exact (3-bit mantissa
times power of two), so the packing adds no error.

Host path (the wall clock is dominated by the ~35MB/s axon tunnel):
upload f16(x) (51MB instead of 102MB — numerically identical, see above),
reuse one cached jitted executable across calls (no per-call retrace),
fetch the 29MB packed result, decode on host.

Accuracy: not bit-exact to the f32 reference — f16(x/p) shifts
round-half-even ties and the f16 maxabs can bump the shared exponent on
~0.04% of blocks. On the fixed harness input: 0.11% of elements differ
by one grid step, L2 rel err 1.042e-2 (gate 2e-2).

Measured per-core device time ~55us/rep (For_i slope); DVE ~34us busy
(tree ~14 + mul ~17 + smalls), ACT ~42us (abs + int8 convert), Pool idle
(DVE and Pool share SBUF ports — any Pool offload is a net loss), DMA
fully overlapped.
"""

import numpy as np

N, C, H, W = 32, 256, 56, 56
NCORES = 8
NPC = N // NCORES        # batches per core
S = H * W                # 3136
BLK = 8
CB = C // BLK            # 32 channel blocks; partition = (n, cb) = 128

LT = 784                 # DMA tile spatial extent (4 tiles)
LC = 784                 # compute chunk width (4 chunks)
BIG_BUFS = 4
EB_BUFS = 3
T_BUFS = 2
R_BUFS = 4
W_BUFS = 4
SMALL_BUFS = 6

_cached = {}


def _build(bench_reps=None):
    import concourse.bacc as bacc
    import concourse.tile as tile
    import concourse.mybir as mybir

    NT = S // LT
    toff = [t * LT for t in range(NT)]
    chunks = []
    for T in range(NT):
        for j in range(LT // LC):
            chunks.append((T, j * LC, LC))
    NCH = len(chunks)

    nc = bacc.Bacc("TRN2", target_bir_lowering=False, debug=False)
    f16, i16, i8 = mybir.dt.float16, mybir.dt.int16, mybir.dt.int8
    Alu, Act = mybir.AluOpType, mybir.ActivationFunctionType

    x_d = nc.dram_tensor("x", [NPC, C, S], f16, kind="ExternalInput").ap()
    q_d = nc.dram_tensor("w8", [NPC, C, S], i8, kind="ExternalOutput").ap()
    p_d = nc.dram_tensor("pb", [NPC, CB, S], f16, kind="ExternalOutput").ap()
    xv = x_d.rearrange("n (cb ch) s -> (n cb) ch s", ch=BLK)
    qv = q_d.rearrange("n (cb ch) s -> (n cb) ch s", ch=BLK)
    pv = p_d.rearrange("n cb s -> (n cb) s")

    with tile.TileContext(nc) as tc:
        with (
            tc.tile_pool(name="big", bufs=BIG_BUFS) as big,
            tc.tile_pool(name="ebp", bufs=EB_BUFS) as ebp,
            tc.tile_pool(name="m4p", bufs=T_BUFS) as m4p,
            tc.tile_pool(name="m2p", bufs=T_BUFS) as m2p,
            tc.tile_pool(name="rp", bufs=R_BUFS) as rp,
            tc.tile_pool(name="wp", bufs=W_BUFS) as wp,
            tc.tile_pool(name="small", bufs=SMALL_BUFS) as small,
        ):
            Xs, ebs, m4s, m2s, mms, pbhs, invhs, r16s, w8s = (
                {} for _ in range(9))

            def xslice(g):
                T, o, w = chunks[g]
                return Xs[T][:, :, o:o + w]

            def st_dma_in(g):
                T, o, w = chunks[g]
                if o == 0:
                    Xs[T] = big.tile([128, BLK, LT], f16, tag="X",
                                     name=f"X{T}")
                    nc.sync.dma_start(Xs[T][:],
                                      xv[:, :, toff[T]:toff[T] + LT])

            def st_abs(g):
                ebs[g] = ebp.tile([128, BLK, LC], f16, tag="eb", name=f"eb{g}")
                nc.scalar.activation(out=ebs[g][:], in_=xslice(g),
                                     func=Act.Abs)

            def st_tree1(g):
                m4s[g] = m4p.tile([128, 4, LC], f16, tag="m4", name=f"m4_{g}")
                nc.vector.tensor_tensor(
                    out=m4s[g][:], in0=ebs[g][:, 0:4, :],
                    in1=ebs[g][:, 4:8, :], op=Alu.max)

            def st_tree2(g):
                m2s[g] = m2p.tile([128, 2, LC], f16, tag="m2", name=f"m2_{g}")
                nc.vector.tensor_tensor(
                    out=m2s[g][:], in0=m4s[g][:, 0:2, :],
                    in1=m4s[g][:, 2:4, :], op=Alu.max)
                del m4s[g], ebs[g]

            def st_tree3(g):
                mms[g] = small.tile([128, LC], f16, tag="mm", name=f"mm{g}")
                nc.vector.tensor_tensor(
                    out=mms[g][:].unsqueeze(1), in0=m2s[g][:, 0:1, :],
                    in1=m2s[g][:, 1:2, :], op=Alu.max)
                del m2s[g]

            def st_pbh(g):
                pbhs[g] = small.tile([128, LC], f16, tag="pbh", name=f"pbh{g}")
                nc.vector.tensor_scalar(
                    out=pbhs[g][:].bitcast(i16), in0=mms[g][:].bitcast(i16),
                    scalar1=0x7C00, scalar2=None, op0=Alu.bitwise_and)
                del mms[g]

            def st_invh(g):
                # invh = 2^-e via bits(0x7800) - bits(pbh); intermediates
                # stay inside int16 range (the engine saturates, it does
                # not wrap). Then invh4 = invh * 4 = 2^(2-e), exact in f16.
                invhs[g] = small.tile([128, LC], f16, tag="invh",
                                      name=f"invh{g}")
                nc.vector.tensor_scalar(
                    out=invhs[g][:].bitcast(i16), in0=pbhs[g][:].bitcast(i16),
                    scalar1=0x7800, scalar2=-1,
                    op0=Alu.subtract, op1=Alu.mult)
                nc.vector.tensor_scalar(
                    out=invhs[g][:], in0=invhs[g][:],
                    scalar1=4.0, scalar2=None, op0=Alu.mult)

            def st_mul(g):
                r16s[g] = rp.tile([128, BLK, LC], f16, tag="r16",
                                  name=f"r16_{g}")
                nc.vector.tensor_tensor(
                    out=r16s[g][:], in0=xslice(g),
                    in1=invhs[g][:].unsqueeze(1).broadcast_to([128, BLK, LC]),
                    op=Alu.mult)
                del invhs[g]

            def st_w(g):
                # ACT Copy with int8 output: the conversion rounds to
                # nearest-even (HW-verified == numpy RNE), so this is the
                # whole round+convert in one ACT pass — off the DVE
                # critical path. r4 in (-8, 8) so values reach at most +-8;
                # the host clips to +-7 during decode (clip commutes with
                # rounding at an integer bound).
                w8s[g] = wp.tile([128, BLK, LC], i8, tag="w8", name=f"w8_{g}")
                if g == NCH - 1:
                    # last chunk on DVE (with fused clip) to balance the
                    # engines: ACT does abs x4 + convert x3 (~37us), DVE
                    # does tree/mul/smalls + this convert (~37us).
                    nc.vector.tensor_scalar(
                        out=w8s[g][:], in0=r16s[g][:],
                        scalar1=-7.0, scalar2=7.0, op0=Alu.max, op1=Alu.min)
                else:
                    nc.scalar.activation(out=w8s[g][:], in_=r16s[g][:],
                                         func=Act.Copy)
                del r16s[g]

            def st_dma_out(g):
                T, o, w = chunks[g]
                lo, hi = toff[T] + o, toff[T] + o + w
                nc.sync.dma_start(qv[:, :, lo:hi], w8s[g][:])
                nc.sync.dma_start(pv[:, lo:hi], pbhs[g][:])
                del w8s[g], pbhs[g]

            stages = [
                [st_dma_in], [st_abs],
                [st_tree1, st_tree2, st_tree3, st_pbh, st_invh, st_mul],
                [st_w], [st_dma_out],
            ]

            def ladder():
                for t in range(NCH + len(stages) - 1):
                    for si, grp in enumerate(stages):
                        g = t - si
                        if 0 <= g < NCH:
                            for fn in grp:
                                fn(g)

            if bench_reps:
                with tc.For_i(0, bench_reps, 1):
                    ladder()
            else:
                ladder()
    nc.compile()
    return nc


def _get_call():
    """Build the Bass module and a reusable jitted sharded executable once.

    run_bass_kernel_spmd re-traces and re-lowers its jax wrapper on every
    call (seconds of host time); building the shard_map jit once and
    re-invoking it keeps warm calls at transfer cost only.
    """
    if "call" in _cached:
        return _cached["call"]

    import jax
    from jax.sharding import Mesh, PartitionSpec, NamedSharding
    from jax.experimental.shard_map import shard_map
    from concourse import mybir
    from concourse.bass2jax import (
        install_neuronx_cc_hook, partition_id_tensor, _bass_exec_p)

    nc = _build()
    install_neuronx_cc_hook()

    partition_name = (nc.partition_id_tensor.name
                      if nc.partition_id_tensor else None)
    in_names, out_names, out_avals, zero_outs = [], [], [], []
    for alloc in nc.m.functions[0].allocations:
        if not isinstance(alloc, mybir.MemoryLocationSet):
            continue
        name = alloc.memorylocations[0].name
        if alloc.kind == "ExternalInput":
            if name != partition_name:
                in_names.append(name)
        elif alloc.kind == "ExternalOutput":
            out_names.append(name)
            shape = tuple(alloc.tensor_shape)
            dtype = mybir.dt.np(alloc.dtype)
            out_avals.append(jax.core.ShapedArray(shape, dtype))
            zero_outs.append(np.zeros(shape, dtype))
    n_params = len(in_names)
    all_in = list(in_names) + list(out_names)
    if partition_name is not None:
        all_in.append(partition_name)

    def _body(*args):
        operands = list(args)
        if partition_name is not None:
            operands.append(partition_id_tensor())
        outs = _bass_exec_p.bind(
            *operands,
            out_avals=tuple(out_avals),
            in_names=tuple(all_in),
            out_names=tuple(out_names),
            lowering_input_output_aliases=(),
            sim_require_finite=True,
            sim_require_nnan=True,
            nc=nc,
        )
        return tuple(outs)

    devices = jax.devices()[:NCORES]
    mesh = Mesh(np.asarray(devices), ("core",))
    in_specs = (PartitionSpec("core"),) * (n_params + len(out_names))
    out_specs = (PartitionSpec("core"),) * len(out_names)
    sharded = jax.jit(
        shard_map(_body, mesh=mesh, in_specs=in_specs, out_specs=out_specs,
                  check_rep=False),
        keep_unused=True,
    )
    shard = NamedSharding(mesh, PartitionSpec("core"))
    concat_zero = [
        jax.device_put(np.zeros((NCORES * z.shape[0], *z.shape[1:]), z.dtype),
                       shard)
        for z in zero_outs
    ]

    def call(xh):
        """xh: np.float16 [N, C, S] -> (w8 [N,C,S] int8, pb [N,CB,S] f16)."""
        dx = jax.device_put(xh, shard)
        outs = sharded(dx, *concat_zero)
        w8 = np.asarray(outs[out_names.index("w8")])
        pb = np.asarray(outs[out_names.index("pb")])
        return w8, pb

    _cached["call"] = call
    return call


def kernel(activations):
    call = _get_call()
    a = np.asarray(activations)
    xh = a.astype(np.float16).reshape(N, C, S)
    w8, pb = call(xh)
    # Exact reconstruction: clip(w8) in [-7,7] times s = p/4 (power of
    # two). The clip finishes the device-side round (which saturates-free
    # produces up to +-8); clip-after-round == round-after-clip here.
    w8 = np.clip(w8, -7, 7)
    scale = pb.astype(np.float32).reshape(N, CB, 1, S) * np.float32(0.25)
    q = np.multiply(w8.reshape(N, CB, BLK, S), scale, dtype=np.float32)
    return q.reshape(N, C, H, W)


# revision 12
# speedup vs baseline: 1.0332x; 1.0332x over previous
"""BFP (block floating point) activation quantization kernel for Trainium2.

Problem: NCHW input [32, 256, 56, 56] f32. Blocks of 8 consecutive channels
share one exponent (at each (n, h, w) position). Per block:
    maxabs = max |x_i|
    p      = 2^floor(log2(maxabs))       (power-of-two part of maxabs)
    s      = p / 4                       (scale; mantissa_bits = 3)
    q_i    = clip(round_half_even(x_i/s), -7, 7) * s

Distribution: batch dim sharded 4 per core across 8 cores; per core the
SBUF partition dim is (n, cb) = 4 batches x 32 channel-blocks = 128, free
dims are (ch in [0,8), spatial chunk).

Device pipeline (all per-element math in f16; exact relative to f16(x)):
    a16  = |f16(x)|              ACT Abs pass (the only ACT use)
    m16  = tree-max over ch      3 packed-f16 DVE TT max passes (4+2+1)
    pbh  = m16 & 0x7C00          f16 power of two = 2^floor(log2(maxabs))
    invh = 2^-e  (bits 0x7800 - pbh), invh4 = 4*invh = 2^(2-e)
    r4   = f16(x) * invh4        f16 TT (== f16(x * 2^(2-e)) exactly:
                                 power-of-2 scaling commutes with rounding)
    w8   = int8(r4)              one pass: the int8 output conversion
                                 rounds to nearest-even (HW-verified on
                                 both engines). 3 of 4 chunks convert on
                                 ACT (Copy), the last on DVE (TS with
                                 fused clip) to balance the engines; the
                                 host clips [-8, 8] to [-7, 7]
Outputs: w8 (int8 mantissas, 25.7MB) + pbh (f16 block scales, 3.2MB).
The host reconstructs q = w8 * (pbh/4) in f32 — exact (3-bit mantissa
times power of two), so the packing adds no error.

Host path (the wall clock is dominated by the ~35MB/s axon tunnel):
upload f16(x) (51MB instead of 102MB — numerically identical, see above),
reuse one cached jitted executable across calls (no per-call retrace),
fetch the 29MB packed result, decode on host.

Accuracy: not bit-exact to the f32 reference — f16(x/p) shifts
round-half-even ties and the f16 maxabs can bump the shared exponent on
~0.04% of blocks. On the fixed harness input: 0.11% of elements differ
by one grid step, L2 rel err 1.042e-2 (gate 2e-2).

Measured per-core device time ~55us/rep (For_i slope); DVE ~34us busy
(tree ~14 + mul ~17 + smalls), ACT ~42us (abs + int8 convert), Pool idle
(DVE and Pool share SBUF ports — any Pool offload is a net loss), DMA
fully overlapped.
"""

import numpy as np

N, C, H, W = 32, 256, 56, 56
NCORES = 8
NPC = N // NCORES        # batches per core
S = H * W                # 3136
BLK = 8
CB = C // BLK            # 32 channel blocks; partition = (n, cb) = 128

LT = 784                 # DMA tile spatial extent (4 tiles)
LC = 784                 # compute chunk width (4 chunks)
BIG_BUFS = 4
EB_BUFS = 3
T_BUFS = 2
R_BUFS = 4
W_BUFS = 4
SMALL_BUFS = 6

_cached = {}


def _build(bench_reps=None):
    import concourse.bacc as bacc
    import concourse.tile as tile
    import concourse.mybir as mybir

    NT = S // LT
    toff = [t * LT for t in range(NT)]
    chunks = []
    for T in range(NT):
        for j in range(LT // LC):
            chunks.append((T, j * LC, LC))
    NCH = len(chunks)

    nc = bacc.Bacc("TRN2", target_bir_lowering=False, debug=False)
    f16, i16, i8 = mybir.dt.float16, mybir.dt.int16, mybir.dt.int8
    Alu, Act = mybir.AluOpType, mybir.ActivationFunctionType

    x_d = nc.dram_tensor("x", [NPC, C, S], f16, kind="ExternalInput").ap()
    q_d = nc.dram_tensor("w8", [NPC, C, S], i8, kind="ExternalOutput").ap()
    p_d = nc.dram_tensor("pb", [NPC, CB, S], f16, kind="ExternalOutput").ap()
    xv = x_d.rearrange("n (cb ch) s -> (n cb) ch s", ch=BLK)
    qv = q_d.rearrange("n (cb ch) s -> (n cb) ch s", ch=BLK)
    pv = p_d.rearrange("n cb s -> (n cb) s")

    with tile.TileContext(nc) as tc:
        with (
            tc.tile_pool(name="big", bufs=BIG_BUFS) as big,
            tc.tile_pool(name="ebp", bufs=EB_BUFS) as ebp,
            tc.tile_pool(name="m4p", bufs=T_BUFS) as m4p,
            tc.tile_pool(name="m2p", bufs=T_BUFS) as m2p,
            tc.tile_pool(name="rp", bufs=R_BUFS) as rp,
            tc.tile_pool(name="wp", bufs=W_BUFS) as wp,
            tc.tile_pool(name="small", bufs=SMALL_BUFS) as small,
        ):
            Xs, ebs, m4s, m2s, mms, pbhs, invhs, r16s, w8s = (
                {} for _ in range(9))

            def xslice(g):
                T, o, w = chunks[g]
                return Xs[T][:, :, o:o + w]

            def st_dma_in(g):
                T, o, w = chunks[g]
                if o == 0:
                    Xs[T] = big.tile([128, BLK, LT], f16, tag="X",
                                     name=f"X{T}")
                    nc.sync.dma_start(Xs[T][:],
                                      xv[:, :, toff[T]:toff[T] + LT])

            def st_abs(g):
                ebs[g] = ebp.tile([128, BLK, LC], f16, tag="eb", name=f"eb{g}")
                nc.scalar.activation(out=ebs[g][:], in_=xslice(g),
                                     func=Act.Abs)

            def st_tree1(g):
                m4s[g] = m4p.tile([128, 4, LC], f16, tag="m4", name=f"m4_{g}")
                nc.vector.tensor_tensor(
                    out=m4s[g][:], in0=ebs[g][:, 0:4, :],
                    in1=ebs[g][:, 4:8, :], op=Alu.max)

            def st_tree2(g):
                m2s[g] = m2p.tile([128, 2, LC], f16, tag="m2", name=f"m2_{g}")
                nc.vector.tensor_tensor(
                    out=m2s[g][:], in0=m4s[g][:, 0:2, :],
                    in1=m4s[g][:, 2:4, :], op=Alu.max)
                del m4s[g], ebs[g]

            def st_tree3(g):
                mms[g] = small.tile([128, LC], f16, tag="mm", name=f"mm{g}")
                nc.vector.tensor_tensor(
                    out=mms[g][:].unsqueeze(1), in0=m2s[g][:, 0:1, :],
                    in1=m2s[g][:, 1:2, :], op=Alu.max)
                del m2s[g]

            def st_pbh(g):
                pbhs[g] = small.tile([128, LC], f16, tag="pbh", name=f"pbh{g}")
                nc.vector.tensor_scalar(
                    out=pbhs[g][:].bitcast(i16), in0=mms[g][:].bitcast(i16),
                    scalar1=0x7C00, scalar2=None, op0=Alu.bitwise_and)
                del mms[g]

            def st_invh(g):
                # invh = 2^-e via bits(0x7800) - bits(pbh); intermediates
                # stay inside int16 range (the engine saturates, it does
                # not wrap). Then invh4 = invh * 4 = 2^(2-e), exact in f16.
                invhs[g] = small.tile([128, LC], f16, tag="invh",
                                      name=f"invh{g}")
                nc.vector.tensor_scalar(
                    out=invhs[g][:].bitcast(i16), in0=pbhs[g][:].bitcast(i16),
                    scalar1=0x7800, scalar2=-1,
                    op0=Alu.subtract, op1=Alu.mult)
                nc.vector.tensor_scalar(
                    out=invhs[g][:], in0=invhs[g][:],
                    scalar1=4.0, scalar2=None, op0=Alu.mult)

            def st_mul(g):
                r16s[g] = rp.tile([128, BLK, LC], f16, tag="r16",
                                  name=f"r16_{g}")
                nc.vector.tensor_tensor(
                    out=r16s[g][:], in0=xslice(g),
                    in1=invhs[g][:].unsqueeze(1).broadcast_to([128, BLK, LC]),
                    op=Alu.mult)
                del invhs[g]

            def st_w(g):
                # ACT Copy with int8 output: the conversion rounds to
                # nearest-even (HW-verified == numpy RNE), so this is the
                # whole round+convert in one ACT pass — off the DVE
                # critical path. r4 in (-8, 8) so values reach at most +-8;
                # the host clips to +-7 during decode (clip commutes with
                # rounding at an integer bound).
                w8s[g] = wp.tile([128, BLK, LC], i8, tag="w8", name=f"w8_{g}")
                if g == NCH - 1:
                    # last chunk on DVE (with fused clip) to balance the
                    # engines: ACT does abs x4 + convert x3 (~37us), DVE
                    # does tree/mul/smalls + this convert (~37us).
                    nc.vector.tensor_scalar(
                        out=w8s[g][:], in0=r16s[g][:],
                        scalar1=-7.0, scalar2=7.0, op0=Alu.max, op1=Alu.min)
                else:
                    nc.scalar.activation(out=w8s[g][:], in_=r16s[g][:],
                                         func=Act.Copy)
                del r16s[g]

            def st_dma_out(g):
                T, o, w = chunks[g]
                lo, hi = toff[T] + o, toff[T] + o + w
                nc.sync.dma_start(qv[:, :, lo:hi], w8s[g][:])
                nc.sync.dma_start(pv[:, lo:hi], pbhs[g][:])
                del w8s[g], pbhs[g]

            stages = [
                [st_dma_in], [st_abs],
                [st_tree1, st_tree2, st_tree3, st_pbh, st_invh, st_mul],
                [st_w], [st_dma_out],
            ]

            def ladder():
                for t in range(NCH + len(stages) - 1):
                    for si, grp in enumerate(stages):
                        g = t - si
                        if 0 <= g < NCH:
                            for fn in grp:
                                fn(g)

            if bench_reps:
                with tc.For_i(0, bench_reps, 1):
                    ladder()
            else:
                ladder()
    nc.compile()
    return nc


def _get_call():
    """Build the Bass module and a reusable jitted sharded executable once.

    run_bass_kernel_spmd re-traces and re-lowers its jax wrapper on every
    call (seconds of host time); building the shard_map jit once and
    re-invoking it keeps warm calls at transfer cost only.
    """
    if "call" in _cached:
        return _cached["call"]

    import jax
    from jax.sharding import Mesh, PartitionSpec, NamedSharding
    from jax.experimental.shard_map import shard_map
    from concourse import mybir
    from concourse.bass2jax import (
        install_neuronx_cc_hook, partition_id_tensor, _bass_exec_p)

    nc = _build()
    install_neuronx_cc_hook()

    partition_name = (nc.partition_id_tensor.name
                      if nc.partition_id_tensor else None)
    in_names, out_names, out_avals, zero_outs = [], [], [], []
    for alloc in nc.m.functions[0].allocations:
        if not isinstance(alloc, mybir.MemoryLocationSet):
            continue
        name = alloc.memorylocations[0].name
        if alloc.kind == "ExternalInput":
            if name != partition_name:
                in_names.append(name)
        elif alloc.kind == "ExternalOutput":
            out_names.append(name)
            shape = tuple(alloc.tensor_shape)
            dtype = mybir.dt.np(alloc.dtype)
            out_avals.append(jax.core.ShapedArray(shape, dtype))
            zero_outs.append(np.zeros(shape, dtype))
    n_params = len(in_names)
    all_in = list(in_names) + list(out_names)
    if partition_name is not None:
        all_in.append(partition_name)

    def _body(*args):
        operands = list(args)
        if partition_name is not None:
            operands.append(partition_id_tensor())
        outs = _bass_exec_p.bind(
            *operands,
            out_avals=tuple(out_avals),
            in_names=tuple(all_in),
            out_names=tuple(out_names),
            lowering_input_output_aliases=(),
            sim_require_finite=True,
            sim_require_nnan=True,
            nc=nc,
        )
        return tuple(outs)

    devices = jax.devices()[:NCORES]
    mesh = Mesh(np.asarray(devices), ("core",))
    in_specs = (PartitionSpec("core"),) * (n_params + len(out_names))
    out_specs = (PartitionSpec("core"),) * len(out_names)
    sharded = jax.jit(
        shard_map(_body, mesh=mesh, in_specs=in_specs, out_specs=out_specs,
                  check_rep=False),
        keep_unused=True,
    )
    shard = NamedSharding(mesh, PartitionSpec("core"))
    concat_zero = [
        jax.device_put(np.zeros((NCORES * z.shape[0], *z.shape[1:]), z.dtype),
                       shard)
        for z in zero_outs
    ]

    def call(xh):
        """xh: np.float16 [N, C, S] -> (w8 [N,C,S] int8, pb [N,CB,S] f16)."""
        dx = jax.device_put(xh, shard)
        outs = sharded(dx, *concat_zero)
        w8 = np.asarray(outs[out_names.index("w8")])
        pb = np.asarray(outs[out_names.index("pb")])
        return w8, pb

    _cached["call"] = call
    return call


def kernel(activations):
    call = _get_call()
    a = np.asarray(activations)
    xh = a.astype(np.float16).reshape(N, C, S)
    w8, pb = call(xh)
    # Exact reconstruction: clip(w8) in [-7,7] times s = p/4 (power of
    # two). The clip finishes the device-side round (which saturates-free
    # produces up to +-8); clip-after-round == round-after-clip here.
    w8 = np.clip(w8, -7, 7)
    scale = pb.astype(np.float32).reshape(N, CB, 1, S) * np.float32(0.25)
    q = np.multiply(w8.reshape(N, CB, BLK, S), scale, dtype=np.float32)
    return q.reshape(N, C, H, W)


# revision 13
# speedup vs baseline: 1.0771x; 1.0425x over previous
"""BFP (block floating point) activation quantization kernel for Trainium2.

Problem: NCHW input [32, 256, 56, 56] f32. Blocks of 8 consecutive channels
share one exponent (at each (n, h, w) position). Per block:
    maxabs = max |x_i|
    p      = 2^floor(log2(maxabs))       (power-of-two part of maxabs)
    s      = p / 4                       (scale; mantissa_bits = 3)
    q_i    = clip(round_half_even(x_i/s), -7, 7) * s

Distribution: batch dim sharded 4 per core across 8 cores; per core the
SBUF partition dim is (n, cb) = 4 batches x 32 channel-blocks = 128, free
dims are (ch in [0,8), spatial chunk).

Device pipeline (all per-element math in f16; exact relative to f16(x)):
    a16  = |f16(x)|              ACT Abs pass (the only ACT use)
    m16  = tree-max over ch      3 packed-f16 DVE TT max passes (4+2+1)
    pbh  = m16 & 0x7C00          f16 power of two = 2^floor(log2(maxabs))
    invh = 2^-e  (bits 0x7800 - pbh), invh4 = 4*invh = 2^(2-e)
    r4   = f16(x) * invh4        f16 TT (== f16(x * 2^(2-e)) exactly:
                                 power-of-2 scaling commutes with rounding)
    w8   = int8(r4)              one pass: the int8 output conversion
                                 rounds to nearest-even (HW-verified on
                                 both engines). 3 of 4 chunks convert on
                                 ACT (Copy), the last on DVE (TS with
                                 fused clip) to balance the engines; the
                                 host clips [-8, 8] to [-7, 7]
Outputs: w8 (int8 mantissas, 25.7MB) + pbh (f16 block scales, 3.2MB).
The host reconstructs q = w8 * (pbh/4) in f32 — exact (3-bit mantissa
times power of two), so the packing adds no error.

Host path (the wall clock is dominated by the ~35MB/s axon tunnel):
upload f16(x) (51MB instead of 102MB — numerically identical, see above),
reuse one cached jitted executable across calls (no per-call retrace),
fetch the 29MB packed result, decode on host.

Accuracy: not bit-exact to the f32 reference — f16(x/p) shifts
round-half-even ties and the f16 maxabs can bump the shared exponent on
~0.04% of blocks. On the fixed harness input: 0.11% of elements differ
by one grid step, L2 rel err 1.042e-2 (gate 2e-2).

Measured per-core device time ~55us/rep (For_i slope); DVE ~34us busy
(tree ~14 + mul ~17 + smalls), ACT ~42us (abs + int8 convert), Pool idle
(DVE and Pool share SBUF ports — any Pool offload is a net loss), DMA
fully overlapped.
"""

import numpy as np

N, C, H, W = 32, 256, 56, 56
NCORES = 8
NPC = N // NCORES        # batches per core
S = H * W                # 3136
BLK = 8
CB = C // BLK            # 32 channel blocks; partition = (n, cb) = 128

LT = 784                 # DMA tile spatial extent (4 tiles)
LC = 784                 # compute chunk width (4 chunks)
BIG_BUFS = 4
EB_BUFS = 3
T_BUFS = 2
R_BUFS = 4
W_BUFS = 4
SMALL_BUFS = 6

_cached = {}


def _build(bench_reps=None):
    import concourse.bacc as bacc
    import concourse.tile as tile
    import concourse.mybir as mybir

    NT = S // LT
    toff = [t * LT for t in range(NT)]
    chunks = []
    for T in range(NT):
        for j in range(LT // LC):
            chunks.append((T, j * LC, LC))
    NCH = len(chunks)

    nc = bacc.Bacc("TRN2", target_bir_lowering=False, debug=False)
    f16, i16, i8 = mybir.dt.float16, mybir.dt.int16, mybir.dt.int8
    Alu, Act = mybir.AluOpType, mybir.ActivationFunctionType

    x_d = nc.dram_tensor("x", [NPC, C, S], f16, kind="ExternalInput").ap()
    q_d = nc.dram_tensor("w8", [NPC, C, S], i8, kind="ExternalOutput").ap()
    p_d = nc.dram_tensor("pb", [NPC, CB, S], f16, kind="ExternalOutput").ap()
    xv = x_d.rearrange("n (cb ch) s -> (n cb) ch s", ch=BLK)
    qv = q_d.rearrange("n (cb ch) s -> (n cb) ch s", ch=BLK)
    pv = p_d.rearrange("n cb s -> (n cb) s")

    with tile.TileContext(nc) as tc:
        with (
            tc.tile_pool(name="big", bufs=BIG_BUFS) as big,
            tc.tile_pool(name="ebp", bufs=EB_BUFS) as ebp,
            tc.tile_pool(name="m4p", bufs=T_BUFS) as m4p,
            tc.tile_pool(name="m2p", bufs=T_BUFS) as m2p,
            tc.tile_pool(name="rp", bufs=R_BUFS) as rp,
            tc.tile_pool(name="wp", bufs=W_BUFS) as wp,
            tc.tile_pool(name="small", bufs=SMALL_BUFS) as small,
        ):
            Xs, ebs, m4s, m2s, mms, pbhs, invhs, r16s, w8s = (
                {} for _ in range(9))

            def xslice(g):
                T, o, w = chunks[g]
                return Xs[T][:, :, o:o + w]

            def st_dma_in(g):
                T, o, w = chunks[g]
                if o == 0:
                    Xs[T] = big.tile([128, BLK, LT], f16, tag="X",
                                     name=f"X{T}")
                    nc.sync.dma_start(Xs[T][:],
                                      xv[:, :, toff[T]:toff[T] + LT])

            def st_abs(g):
                ebs[g] = ebp.tile([128, BLK, LC], f16, tag="eb", name=f"eb{g}")
                nc.scalar.activation(out=ebs[g][:], in_=xslice(g),
                                     func=Act.Abs)

            def st_tree1(g):
                m4s[g] = m4p.tile([128, 4, LC], f16, tag="m4", name=f"m4_{g}")
                nc.vector.tensor_tensor(
                    out=m4s[g][:], in0=ebs[g][:, 0:4, :],
                    in1=ebs[g][:, 4:8, :], op=Alu.max)

            def st_tree2(g):
                m2s[g] = m2p.tile([128, 2, LC], f16, tag="m2", name=f"m2_{g}")
                nc.vector.tensor_tensor(
                    out=m2s[g][:], in0=m4s[g][:, 0:2, :],
                    in1=m4s[g][:, 2:4, :], op=Alu.max)
                del m4s[g], ebs[g]

            def st_tree3(g):
                mms[g] = small.tile([128, LC], f16, tag="mm", name=f"mm{g}")
                nc.vector.tensor_tensor(
                    out=mms[g][:].unsqueeze(1), in0=m2s[g][:, 0:1, :],
                    in1=m2s[g][:, 1:2, :], op=Alu.max)
                del m2s[g]

            def st_pbh(g):
                pbhs[g] = small.tile([128, LC], f16, tag="pbh", name=f"pbh{g}")
                nc.vector.tensor_scalar(
                    out=pbhs[g][:].bitcast(i16), in0=mms[g][:].bitcast(i16),
                    scalar1=0x7C00, scalar2=None, op0=Alu.bitwise_and)
                del mms[g]

            def st_pbh_dma(g):
                # pbh is final here — DMA it out now so the transfer
                # overlaps compute instead of extending the drain tail.
                T, o, w = chunks[g]
                lo = toff[T] + o
                nc.sync.dma_start(pv[:, lo:lo + w], pbhs[g][:])

            def st_invh(g):
                # invh = 2^-e via bits(0x7800) - bits(pbh); intermediates
                # stay inside int16 range (the engine saturates, it does
                # not wrap). Then invh4 = invh * 4 = 2^(2-e), exact in f16.
                invhs[g] = small.tile([128, LC], f16, tag="invh",
                                      name=f"invh{g}")
                nc.vector.tensor_scalar(
                    out=invhs[g][:].bitcast(i16), in0=pbhs[g][:].bitcast(i16),
                    scalar1=0x7800, scalar2=-1,
                    op0=Alu.subtract, op1=Alu.mult)
                nc.vector.tensor_scalar(
                    out=invhs[g][:], in0=invhs[g][:],
                    scalar1=4.0, scalar2=None, op0=Alu.mult)

            def st_mul(g):
                r16s[g] = rp.tile([128, BLK, LC], f16, tag="r16",
                                  name=f"r16_{g}")
                nc.vector.tensor_tensor(
                    out=r16s[g][:], in0=xslice(g),
                    in1=invhs[g][:].unsqueeze(1).broadcast_to([128, BLK, LC]),
                    op=Alu.mult)
                del invhs[g]

            def st_w(g):
                # ACT Copy with int8 output: the conversion rounds to
                # nearest-even (HW-verified == numpy RNE), so this is the
                # whole round+convert in one ACT pass — off the DVE
                # critical path. r4 in (-8, 8) so values reach at most +-8;
                # the host clips to +-7 during decode (clip commutes with
                # rounding at an integer bound).
                w8s[g] = wp.tile([128, BLK, LC], i8, tag="w8", name=f"w8_{g}")
                if g == NCH - 1:
                    # last chunk on DVE (with fused clip) to balance the
                    # engines: ACT does abs x4 + convert x3 (~37us), DVE
                    # does tree/mul/smalls + this convert (~37us).
                    nc.vector.tensor_scalar(
                        out=w8s[g][:], in0=r16s[g][:],
                        scalar1=-7.0, scalar2=7.0, op0=Alu.max, op1=Alu.min)
                else:
                    nc.scalar.activation(out=w8s[g][:], in_=r16s[g][:],
                                         func=Act.Copy)
                del r16s[g]

            def st_dma_out(g):
                T, o, w = chunks[g]
                lo, hi = toff[T] + o, toff[T] + o + w
                nc.sync.dma_start(qv[:, :, lo:hi], w8s[g][:])
                del w8s[g], pbhs[g]

            stages = [
                [st_dma_in], [st_abs],
                [st_tree1, st_tree2, st_tree3, st_pbh, st_invh, st_pbh_dma,
                 st_mul],
                [st_w], [st_dma_out],
            ]

            def ladder():
                for t in range(NCH + len(stages) - 1):
                    for si, grp in enumerate(stages):
                        g = t - si
                        if 0 <= g < NCH:
                            for fn in grp:
                                fn(g)

            if bench_reps:
                with tc.For_i(0, bench_reps, 1):
                    ladder()
            else:
                ladder()
    nc.compile()
    return nc


def _get_call():
    """Build the Bass module and a reusable jitted sharded executable once.

    run_bass_kernel_spmd re-traces and re-lowers its jax wrapper on every
    call (seconds of host time); building the shard_map jit once and
    re-invoking it keeps warm calls at transfer cost only.
    """
    if "call" in _cached:
        return _cached["call"]

    import jax
    from jax.sharding import Mesh, PartitionSpec, NamedSharding
    from jax.experimental.shard_map import shard_map
    from concourse import mybir
    from concourse.bass2jax import (
        install_neuronx_cc_hook, partition_id_tensor, _bass_exec_p)

    nc = _build()
    install_neuronx_cc_hook()

    partition_name = (nc.partition_id_tensor.name
                      if nc.partition_id_tensor else None)
    in_names, out_names, out_avals, zero_outs = [], [], [], []
    for alloc in nc.m.functions[0].allocations:
        if not isinstance(alloc, mybir.MemoryLocationSet):
            continue
        name = alloc.memorylocations[0].name
        if alloc.kind == "ExternalInput":
            if name != partition_name:
                in_names.append(name)
        elif alloc.kind == "ExternalOutput":
            out_names.append(name)
            shape = tuple(alloc.tensor_shape)
            dtype = mybir.dt.np(alloc.dtype)
            out_avals.append(jax.core.ShapedArray(shape, dtype))
            zero_outs.append(np.zeros(shape, dtype))
    n_params = len(in_names)
    all_in = list(in_names) + list(out_names)
    if partition_name is not None:
        all_in.append(partition_name)

    def _body(*args):
        operands = list(args)
        if partition_name is not None:
            operands.append(partition_id_tensor())
        outs = _bass_exec_p.bind(
            *operands,
            out_avals=tuple(out_avals),
            in_names=tuple(all_in),
            out_names=tuple(out_names),
            lowering_input_output_aliases=(),
            sim_require_finite=True,
            sim_require_nnan=True,
            nc=nc,
        )
        return tuple(outs)

    devices = jax.devices()[:NCORES]
    mesh = Mesh(np.asarray(devices), ("core",))
    in_specs = (PartitionSpec("core"),) * (n_params + len(out_names))
    out_specs = (PartitionSpec("core"),) * len(out_names)
    sharded = jax.jit(
        shard_map(_body, mesh=mesh, in_specs=in_specs, out_specs=out_specs,
                  check_rep=False),
        keep_unused=True,
    )
    shard = NamedSharding(mesh, PartitionSpec("core"))
    concat_zero = [
        jax.device_put(np.zeros((NCORES * z.shape[0], *z.shape[1:]), z.dtype),
                       shard)
        for z in zero_outs
    ]

    def call(xh):
        """xh: np.float16 [N, C, S] -> (w8 [N,C,S] int8, pb [N,CB,S] f16)."""
        dx = jax.device_put(xh, shard)
        outs = sharded(dx, *concat_zero)
        w8 = np.asarray(outs[out_names.index("w8")])
        pb = np.asarray(outs[out_names.index("pb")])
        return w8, pb

    _cached["call"] = call
    return call


def kernel(activations):
    call = _get_call()
    a = np.asarray(activations)
    xh = a.astype(np.float16).reshape(N, C, S)
    w8, pb = call(xh)
    # Exact reconstruction: clip(w8) in [-7,7] times s = p/4 (power of
    # two). The clip finishes the device-side round (which saturates-free
    # produces up to +-8); clip-after-round == round-after-clip here.
    w8 = np.clip(w8, -7, 7)
    scale = pb.astype(np.float32).reshape(N, CB, 1, S) * np.float32(0.25)
    q = np.multiply(w8.reshape(N, CB, BLK, S), scale, dtype=np.float32)
    return q.reshape(N, C, H, W)
